# revision 1
# baseline (speedup 1.0000x reference)
"""Trainium2 Bass kernel for nn_AttentionFlowLayer (trilinear similarity).

Reference math (per batch b):
    S[t, j] = (H[t] * w3) . U[j]  +  H[t] . w1  +  U[j] . w2

Sharding: data-parallel over batch — 8 batches, one per NeuronCore.
Per-core: S_b = Hw @ U^T + s_h 1^T + 1 s_u^T, computed as a single PSUM
accumulation: a K=128 matmul (f32r, full PE rate) plus a K=2 rank-2
augmented matmul with E = [s_h; 1], F = [1; s_u].

Self-contained: hardcodes shapes B=8, T=J=2048, D=128, fp32.
"""

import numpy as np

import concourse.bass as bass  # noqa: F401  (bass types used via bacc/tile)
import concourse.mybir as mybir
import concourse.tile as tile
from concourse import bacc
from concourse.bass_utils import run_bass_kernel_spmd
from concourse.masks import make_identity

F32 = mybir.dt.float32
F32R = mybir.dt.float32r

B = 8          # batch -> one per core
T = 2048       # rows of S (t) and columns (j)
D = 128        # feature dim = contraction K = partition count
P = 128        # partitions / tile edge
NT = T // P    # 16 row tiles
JBW = 512      # PSUM bank width in fp32 -> matmul N
NJ = T // JBW  # 4 j banks

_NC_CACHE = {}


def _build_nc():
    nc = bacc.Bacc(
        "TRN2",
        target_bir_lowering=False,
        debug=False,
        num_devices=B,
    )
    H = nc.dram_tensor("H", [T, D], F32, kind="ExternalInput").ap()
    U = nc.dram_tensor("U", [T, D], F32, kind="ExternalInput").ap()
    w = nc.dram_tensor("weight", [3 * D], F32, kind="ExternalInput").ap()
    S = nc.dram_tensor("S", [T, T], F32, kind="ExternalOutput").ap()

    with tile.TileContext(nc) as tc:
        with tc.tile_pool(name="persist", bufs=1) as pp:
            ident = pp.tile([P, P], F32)
            make_identity(nc, ident[:])

            # Inputs, natural layout: [p, ti, d] with t = ti*128 + p
            H_sb = pp.tile([P, NT, D], F32)
            nc.sync.dma_start(out=H_sb[:], in_=H.rearrange("(ti p) d -> p ti d", p=P))
            U_sb = pp.tile([P, NT, D], F32)
            nc.sync.dma_start(out=U_sb[:], in_=U.rearrange("(ti p) d -> p ti d", p=P))

            # weight as a row on partition 0, then per-d columns via rank-1 matmul
            w_row = pp.tile([1, 3 * D], F32)
            nc.sync.dma_start(out=w_row[:], in_=w.unsqueeze(0))

            # Transposed operands (d on partitions), rounded to f32r for the PE
            HT = pp.tile([P, T], F32R)    # H^T (unscaled, for s_h)
            HwT = pp.tile([P, T], F32R)   # (H * w3)^T
            UT = pp.tile([P, T], F32R)    # U^T
            # Rank-2 bias factors: E = [s_h; 1] (2 x T), F = [1; s_u] (2 x T)
            Es = pp.tile([2, T], F32R)
            Fs = pp.tile([2, T], F32R)

            with (
                tc.tile_pool(name="psum_tr", bufs=4, space="PSUM") as psum_tr,
                tc.tile_pool(name="psum_sm", bufs=2, space="PSUM") as psum_sm,
                tc.tile_pool(name="prolog", bufs=2) as prolog,
            ):
                one_cell = prolog.tile([1, 1], F32)
                nc.vector.memset(one_cell[:], 1.0)
                ones_f32 = prolog.tile([1, T], F32)
                nc.vector.memset(ones_f32[:], 1.0)
                ones_row = prolog.tile([1, T], F32R)
                nc.vector.tensor_copy(ones_row[:], ones_f32[:])

                # w columns: psum[d, 1] = w_row_chunk^T (rank-1 with scalar 1)
                wcol_ps = psum_sm.tile([P, 3], F32)
                for k in range(3):
                    nc.tensor.matmul(
                        wcol_ps[:, k : k + 1],
                        w_row[0:1, k * D : (k + 1) * D],
                        one_cell[:],
                        start=True,
                        stop=True,
                    )
                w12_r = prolog.tile([P, 2], F32R)   # w1, w2 columns (f32r lhsT)
                nc.scalar.copy(w12_r[:], wcol_ps[:, 0:2])
                w3col = prolog.tile([P, 1], F32)    # w3 column (fp32 scale operand)
                nc.scalar.copy(w3col[:], wcol_ps[:, 2:3])

                # Transpose all H and U tiles; build HT / HwT / UT
                for ti in range(NT):
                    csl = slice(ti * P, (ti + 1) * P)
                    h_ps = psum_tr.tile([P, P], F32, tag="tr")
                    nc.tensor.transpose(h_ps[:], H_sb[:, ti, :], ident[:])
                    nc.scalar.copy(HT[:, csl], h_ps[:])
                    nc.vector.tensor_scalar_mul(HwT[:, csl], h_ps[:], w3col[:])
                    u_ps = psum_tr.tile([P, P], F32, tag="tr")
                    nc.tensor.transpose(u_ps[:], U_sb[:, ti, :], ident[:])
                    nc.scalar.copy(UT[:, csl], u_ps[:])

                # s_h = w1 . H^T -> Es row 0;  s_u = w2 . U^T -> staged row
                su_row = prolog.tile([1, T], F32R)
                for jb in range(NJ):
                    jsl = slice(jb * JBW, (jb + 1) * JBW)
                    sh_ps = psum_sm.tile([1, JBW], F32, tag="sv")
                    nc.tensor.matmul(
                        sh_ps[:], w12_r[:, 0:1], HT[:, jsl], start=True, stop=True
                    )
                    nc.vector.tensor_copy(Es[0:1, jsl], sh_ps[:])
                    su_ps = psum_sm.tile([1, JBW], F32, tag="sv")
                    nc.tensor.matmul(
                        su_ps[:], w12_r[:, 1:2], UT[:, jsl], start=True, stop=True
                    )
                    nc.vector.tensor_copy(su_row[0:1, jsl], su_ps[:])

                # Row 1 of each stack (partition 1 is not engine-addressable;
                # write it via SBUF->SBUF DMA)
                nc.sync.dma_start(out=Es[1:2, :], in_=ones_row[:])
                nc.vector.tensor_copy(Fs[0:1, :], ones_row[:])
                nc.sync.dma_start(out=Fs[1:2, :], in_=su_row[:])

            # Main loop: 16 row-tiles x 4 psum banks; two matmuls per bank
            # (K=128 product term + K=2 rank-2 bias), then PSUM->SBUF copy
            # split across ScalarE/VectorE, then one 1 MiB DMA per row-tile.
            with (
                tc.tile_pool(name="psum_mm", bufs=8, space="PSUM") as psum_mm,
                tc.tile_pool(name="outp", bufs=3) as outp,
            ):
                for ti in range(NT):
                    tsl = slice(ti * P, (ti + 1) * P)
                    out_sb = outp.tile([P, T], F32)
                    for jb in range(NJ):
                        jsl = slice(jb * JBW, (jb + 1) * JBW)
                        ps = psum_mm.tile([P, JBW], F32, tag="mm")
                        nc.tensor.matmul(
                            ps[:], HwT[:, tsl], UT[:, jsl], start=True, stop=False
                        )
                        nc.tensor.matmul(
                            ps[:], Es[:, tsl], Fs[:, jsl], start=False, stop=True
                        )
                        if jb % 2 == 0:
                            nc.scalar.copy(out_sb[:, jsl], ps[:])
                        else:
                            nc.vector.tensor_copy(out_sb[:, jsl], ps[:])
                    nc.sync.dma_start(out=S[tsl, :], in_=out_sb[:])

    nc.compile()
    return nc


def _get_nc():
    if "nc" not in _NC_CACHE:
        _NC_CACHE["nc"] = _build_nc()
    return _NC_CACHE["nc"]


def kernel_with_results(H, U, weight, trace=False):
    assert H.shape == (B, T, D) and U.shape == (B, T, D)
    assert weight.shape == (3 * D,)
    nc = _get_nc()
    in_maps = [
        {
            "H": np.ascontiguousarray(H[b], dtype=np.float32),
            "U": np.ascontiguousarray(U[b], dtype=np.float32),
            "weight": np.ascontiguousarray(weight, dtype=np.float32),
        }
        for b in range(B)
    ]
    res = run_bass_kernel_spmd(nc, in_maps, list(range(B)), trace=trace)
    out = np.stack([res.results[b]["S"] for b in range(B)], axis=0)
    return out, res


def kernel(H, U, weight):
    out, _ = kernel_with_results(H, U, weight)
    return out


if __name__ == "__main__":
    rng = np.random.default_rng(0)
    H = rng.standard_normal((B, T, D)).astype(np.float32)
    U = rng.standard_normal((B, T, D)).astype(np.float32)
    w = rng.random(3 * D).astype(np.float32)
    out = kernel(H, U, w)
    print(out.shape, out.dtype)


# revision 4
# speedup vs baseline: 1.4178x; 1.4178x over previous
"""Trainium2 Bass kernel for nn_AttentionFlowLayer (trilinear similarity).

Reference math (per batch b):
    S[t, j] = (H[t] * w3) . U[j]  +  H[t] . w1  +  U[j] . w2

Sharding: data-parallel over batch — 8 batches, one per NeuronCore.
Per-core: S_b = Hw @ U^T + s_h 1^T + 1 s_u^T, computed as a single PSUM
accumulation: a K=128 matmul (f32r, full PE rate) plus a K=2 rank-2
augmented matmul with E = [s_h; 1], F = [1; s_u].

Self-contained: hardcodes shapes B=8, T=J=2048, D=128, fp32.
"""

import numpy as np

import concourse.bass as bass  # noqa: F401  (bass types used via bacc/tile)
import concourse.mybir as mybir
import concourse.tile as tile
from concourse import bacc
from concourse.bass_utils import run_bass_kernel_spmd
from concourse.masks import make_identity

F32 = mybir.dt.float32
F32R = mybir.dt.float32r

B = 8          # batch -> one per core
T = 2048       # rows of S (t) and columns (j)
D = 128        # feature dim = contraction K = partition count
P = 128        # partitions / tile edge
NT = T // P    # 16 row tiles
JBW = 512      # PSUM bank width in fp32 -> matmul N
NJ = T // JBW  # 4 j banks

_NC_CACHE = {}


def _build_nc():
    nc = bacc.Bacc(
        "TRN2",
        target_bir_lowering=False,
        debug=False,
        num_devices=B,
    )
    H = nc.dram_tensor("H", [T, D], F32, kind="ExternalInput").ap()
    U = nc.dram_tensor("U", [T, D], F32, kind="ExternalInput").ap()
    w = nc.dram_tensor("weight", [3 * D], F32, kind="ExternalInput").ap()
    S = nc.dram_tensor("S", [T, T], F32, kind="ExternalOutput").ap()

    with tile.TileContext(nc) as tc:
        with tc.tile_pool(name="persist", bufs=1) as pp:
            ident = pp.tile([P, P], F32)
            make_identity(nc, ident[:])

            # Inputs, natural layout: [p, ti, d] with t = ti*128 + p
            H_sb = pp.tile([P, NT, D], F32)
            nc.sync.dma_start(out=H_sb[:], in_=H.rearrange("(ti p) d -> p ti d", p=P))
            U_sb = pp.tile([P, NT, D], F32)
            nc.sync.dma_start(out=U_sb[:], in_=U.rearrange("(ti p) d -> p ti d", p=P))

            # weight as a row on partition 0, then per-d columns via rank-1 matmul
            w_row = pp.tile([1, 3 * D], F32)
            nc.sync.dma_start(out=w_row[:], in_=w.unsqueeze(0))

            # Transposed operands (d on partitions), rounded to f32r for the PE
            HT = pp.tile([P, T], F32R)    # H^T (unscaled, for s_h)
            HwT = pp.tile([P, T], F32R)   # (H * w3)^T
            UT = pp.tile([P, T], F32R)    # U^T
            # Rank-2 bias factors padded to K=128 (a K=2 matmul measures ~1.07us
            # on HW vs ~0.26us for a full K=128 one): E = [s_h; 1; 0...],
            # F = [1; s_u; 0...]
            Es = pp.tile([P, T], F32R)
            Fs = pp.tile([P, T], F32R)

            with (
                tc.tile_pool(name="psum_tr", bufs=4, space="PSUM") as psum_tr,
                tc.tile_pool(name="psum_sm", bufs=2, space="PSUM") as psum_sm,
                tc.tile_pool(name="prolog", bufs=1) as prolog,
            ):
                one_cell = prolog.tile([1, 1], F32)
                nc.vector.memset(one_cell[:], 1.0)
                ones_f32 = prolog.tile([1, T], F32)
                nc.vector.memset(ones_f32[:], 1.0)
                ones_row = prolog.tile([1, T], F32R)
                nc.vector.tensor_copy(ones_row[:], ones_f32[:])
                # zero-fill the padded bias factors (memset can't write f32r;
                # round-copy a zeroed f32 staging tile instead)
                zero_f32 = prolog.tile([P, T], F32)
                nc.vector.memset(zero_f32[:], 0.0)
                nc.vector.tensor_copy(Es[:], zero_f32[:])
                nc.scalar.copy(Fs[:], zero_f32[:])

                # w columns: psum[d, 1] = w_row_chunk^T (rank-1 with scalar 1)
                wcol_ps = psum_sm.tile([P, 3], F32)
                for k in range(3):
                    nc.tensor.matmul(
                        wcol_ps[:, k : k + 1],
                        w_row[0:1, k * D : (k + 1) * D],
                        one_cell[:],
                        start=True,
                        stop=True,
                    )
                w12_r = prolog.tile([P, 2], F32R)   # w1, w2 columns (f32r lhsT)
                nc.scalar.copy(w12_r[:], wcol_ps[:, 0:2])
                w3col = prolog.tile([P, 1], F32)    # w3 column (fp32 scale operand)
                nc.scalar.copy(w3col[:], wcol_ps[:, 2:3])

                # Transpose all H and U tiles; build HT / HwT / UT
                for ti in range(NT):
                    csl = slice(ti * P, (ti + 1) * P)
                    h_ps = psum_tr.tile([P, P], F32, tag="tr")
                    nc.tensor.transpose(h_ps[:], H_sb[:, ti, :], ident[:])
                    nc.scalar.copy(HT[:, csl], h_ps[:])
                    nc.vector.tensor_scalar_mul(HwT[:, csl], h_ps[:], w3col[:])
                    u_ps = psum_tr.tile([P, P], F32, tag="tr")
                    nc.tensor.transpose(u_ps[:], U_sb[:, ti, :], ident[:])
                    nc.scalar.copy(UT[:, csl], u_ps[:])

                # s_h = w1 . H^T -> Es row 0;  s_u = w2 . U^T -> staged row
                su_row = prolog.tile([1, T], F32R)
                for jb in range(NJ):
                    jsl = slice(jb * JBW, (jb + 1) * JBW)
                    sh_ps = psum_sm.tile([1, JBW], F32, tag="sv")
                    nc.tensor.matmul(
                        sh_ps[:], w12_r[:, 0:1], HT[:, jsl], start=True, stop=True
                    )
                    nc.vector.tensor_copy(Es[0:1, jsl], sh_ps[:])
                    su_ps = psum_sm.tile([1, JBW], F32, tag="sv")
                    nc.tensor.matmul(
                        su_ps[:], w12_r[:, 1:2], UT[:, jsl], start=True, stop=True
                    )
                    nc.vector.tensor_copy(su_row[0:1, jsl], su_ps[:])

                # Row 1 of each stack (partition 1 is not engine-addressable;
                # write it via SBUF->SBUF DMA)
                nc.sync.dma_start(out=Es[1:2, :], in_=ones_row[:])
                nc.vector.tensor_copy(Fs[0:1, :], ones_row[:])
                nc.sync.dma_start(out=Fs[1:2, :], in_=su_row[:])

            # Main loop: 16 row-tiles x 4 psum banks; two matmuls per bank
            # (K=128 product term + K=2 rank-2 bias), then PSUM->SBUF copy
            # split across ScalarE/VectorE, then one 1 MiB DMA per row-tile.
            with (
                tc.tile_pool(name="psum_mm", bufs=8, space="PSUM") as psum_mm,
                tc.tile_pool(name="outp", bufs=3) as outp,
            ):
                for ti in range(NT):
                    tsl = slice(ti * P, (ti + 1) * P)
                    out_sb = outp.tile([P, T], F32)
                    for jb in range(NJ):
                        jsl = slice(jb * JBW, (jb + 1) * JBW)
                        ps = psum_mm.tile([P, JBW], F32, tag="mm")
                        nc.tensor.matmul(
                            ps[:], HwT[:, tsl], UT[:, jsl], start=True, stop=False
                        )
                        nc.tensor.matmul(
                            ps[:], Es[:, tsl], Fs[:, jsl], start=False, stop=True
                        )
                        if jb % 2 == 0:
                            nc.scalar.copy(out_sb[:, jsl], ps[:])
                        else:
                            nc.vector.tensor_copy(out_sb[:, jsl], ps[:])
                    nc.sync.dma_start(out=S[tsl, :], in_=out_sb[:])

    nc.compile()
    return nc


def _get_nc():
    if "nc" not in _NC_CACHE:
        _NC_CACHE["nc"] = _build_nc()
    return _NC_CACHE["nc"]


def kernel_with_results(H, U, weight, trace=False):
    assert H.shape == (B, T, D) and U.shape == (B, T, D)
    assert weight.shape == (3 * D,)
    nc = _get_nc()
    in_maps = [
        {
            "H": np.ascontiguousarray(H[b], dtype=np.float32),
            "U": np.ascontiguousarray(U[b], dtype=np.float32),
            "weight": np.ascontiguousarray(weight, dtype=np.float32),
        }
        for b in range(B)
    ]
    res = run_bass_kernel_spmd(nc, in_maps, list(range(B)), trace=trace)
    out = np.stack([res.results[b]["S"] for b in range(B)], axis=0)
    return out, res


def kernel(H, U, weight):
    out, _ = kernel_with_results(H, U, weight)
    return out


if __name__ == "__main__":
    rng = np.random.default_rng(0)
    H = rng.standard_normal((B, T, D)).astype(np.float32)
    U = rng.standard_normal((B, T, D)).astype(np.float32)
    w = rng.random(3 * D).astype(np.float32)
    out = kernel(H, U, w)
    print(out.shape, out.dtype)


# revision 5
# speedup vs baseline: 1.7285x; 1.2192x over previous
"""Trainium2 Bass kernel for nn_AttentionFlowLayer (trilinear similarity).

Reference math (per batch b):
    S[t, j] = (H[t] * w3) . U[j]  +  H[t] . w1  +  U[j] . w2

Folded form used here: with U'[j, d] = w3[d] * U[j, d] + w1[d] and
s_u[j] = U[j] . w2,

    S^T[j, t] = sum_d U'[j, d] * H[t, d]  +  s_u[j]

so each 128x512 output tile of S^T needs ONE f32r matmul
(lhsT = U'^T chunk, rhs = H^T chunk) and the s_u bias is per-partition,
folded for free into the PSUM->SBUF copy (ScalarE activation-bias /
VectorE tensor_scalar add). The kernel writes S^T per batch; the host
transposes on gather.

Sharding: data-parallel over batch - 8 batches, one per NeuronCore.
Self-contained: hardcodes shapes B=8, T=J=2048, D=128, fp32.
"""

import numpy as np

import concourse.mybir as mybir
import concourse.tile as tile
from concourse import bacc
from concourse.bass_utils import run_bass_kernel_spmd
from concourse.masks import make_identity

F32 = mybir.dt.float32
F32R = mybir.dt.float32r
IDENT = mybir.ActivationFunctionType.Identity

B = 8          # batch -> one per core
T = 2048       # rows of S (t) and columns (j)
D = 128        # feature dim = contraction K
P = 128        # partitions / tile edge
NT = T // P    # 16 tiles per side
TCW = 512      # PSUM bank width in fp32 -> matmul N
NTC = T // TCW  # 4 t chunks per output row-block

U_LEAD = 3     # U-transpose lead distance ahead of the main loop

_NC_CACHE = {}


def _build_nc():
    nc = bacc.Bacc(
        "TRN2",
        target_bir_lowering=False,
        debug=False,
        num_devices=B,
    )
    H = nc.dram_tensor("H", [T, D], F32, kind="ExternalInput").ap()
    U = nc.dram_tensor("U", [T, D], F32, kind="ExternalInput").ap()
    w = nc.dram_tensor("weight", [3 * D], F32, kind="ExternalInput").ap()
    # Holds S^T for this batch; host transposes after gather.
    S = nc.dram_tensor("S", [T, T], F32, kind="ExternalOutput").ap()

    with tile.TileContext(nc) as tc:
        with (
            tc.tile_pool(name="persist", bufs=1) as pp,
            tc.tile_pool(name="tmp", bufs=3) as tmp,
            tc.tile_pool(name="psum_tr", bufs=2, space="PSUM") as psum_tr,
            tc.tile_pool(name="psum_sm", bufs=2, space="PSUM") as psum_sm,
            tc.tile_pool(name="psum_mm", bufs=4, space="PSUM") as psum_mm,
            tc.tile_pool(name="outp", bufs=3) as outp,
        ):
            ident = pp.tile([P, P], F32)
            make_identity(nc, ident[:])

            # Inputs, natural layout [p, ti, d] (t = ti*128 + p), chunked DMAs
            # so the first transposes can start early.
            H_sb = pp.tile([P, NT, D], F32)
            U_sb = pp.tile([P, NT, D], F32)
            H_r = H.rearrange("(ti p) d -> p ti d", p=P)
            U_r = U.rearrange("(ti p) d -> p ti d", p=P)
            for c in range(4):
                csl = slice(4 * c, 4 * c + 4)
                nc.sync.dma_start(out=U_sb[:, csl, :], in_=U_r[:, csl, :])
                nc.sync.dma_start(out=H_sb[:, csl, :], in_=H_r[:, csl, :])

            w_row = pp.tile([1, 3 * D], F32)
            nc.sync.dma_start(out=w_row[:], in_=w.unsqueeze(0))
            one_cell = pp.tile([1, 1], F32)
            nc.vector.memset(one_cell[:], 1.0)

            # weight columns [d, 1] via rank-1 matmuls (w_row_chunk^T x 1)
            wcol_ps = psum_sm.tile([P, 3], F32, tag="sv")
            for k in range(3):
                nc.tensor.matmul(
                    wcol_ps[:, k : k + 1],
                    w_row[0:1, k * D : (k + 1) * D],
                    one_cell[:],
                    start=True,
                    stop=True,
                )
            wcol = pp.tile([P, 3], F32)
            nc.scalar.copy(wcol[:], wcol_ps[:])
            w1col = wcol[:, 0:1]
            w2col = wcol[:, 1:2]
            w3col = wcol[:, 2:3]

            # Persistent transposed operands (d on partitions), f32r for PE
            HT = pp.tile([P, T], F32R)     # H^T
            UpT = pp.tile([P, T], F32R)    # U'^T = w3 * U^T + w1
            s_u_col = pp.tile([P, NT], F32)  # s_u, one 128-column per j-tile

            # H transposes first: every output row-block needs full H^T.
            for ti in range(NT):
                csl = slice(ti * P, (ti + 1) * P)
                h_ps = psum_tr.tile([P, P], F32, tag="tr")
                nc.tensor.transpose(h_ps[:], H_sb[:, ti, :], ident[:])
                if ti % 2 == 0:
                    nc.scalar.copy(HT[:, csl], h_ps[:])
                else:
                    nc.vector.tensor_copy(HT[:, csl], h_ps[:])

            def do_u(k):
                csl = slice(k * P, (k + 1) * P)
                u_ps = psum_tr.tile([P, P], F32, tag="tr")
                nc.tensor.transpose(u_ps[:], U_sb[:, k, :], ident[:])
                # U'^T chunk = w3 * U^T + w1, rounded to f32r
                nc.vector.tensor_scalar(
                    UpT[:, csl], u_ps[:], w3col, w1col,
                    op0=mybir.AluOpType.mult, op1=mybir.AluOpType.add,
                )
                # unscaled U^T chunk (fp32) for the s_u column
                ut_tmp = tmp.tile([P, P], F32, tag="ut")
                nc.scalar.copy(ut_tmp[:], u_ps[:])
                su_ps = psum_sm.tile([P, 1], F32, tag="sv")
                nc.tensor.matmul(su_ps[:], ut_tmp[:], w2col, start=True, stop=True)
                nc.scalar.copy(s_u_col[:, k : k + 1], su_ps[:])

            for k in range(U_LEAD):
                do_u(k)

            # Main loop: output row-block jt (128 j's x full t), 4 psum banks
            for jt in range(NT):
                if jt + U_LEAD < NT:
                    do_u(jt + U_LEAD)
                jsl = slice(jt * P, (jt + 1) * P)
                su_b = s_u_col[:, jt : jt + 1]
                out_sb = outp.tile([P, T], F32)
                for tc_i in range(NTC):
                    tsl = slice(tc_i * TCW, (tc_i + 1) * TCW)
                    ps = psum_mm.tile([P, TCW], F32, tag="mm")
                    nc.tensor.matmul(
                        ps[:], UpT[:, jsl], HT[:, tsl], start=True, stop=True
                    )
                    if tc_i % 2 == 0:
                        nc.scalar.activation(
                            out_sb[:, tsl], ps[:], IDENT, bias=su_b, scale=1.0
                        )
                    else:
                        nc.vector.tensor_scalar_add(out_sb[:, tsl], ps[:], su_b)
                nc.sync.dma_start(out=S[jsl, :], in_=out_sb[:])

    nc.compile()
    return nc


def _get_nc():
    if "nc" not in _NC_CACHE:
        _NC_CACHE["nc"] = _build_nc()
    return _NC_CACHE["nc"]


def kernel_with_results(H, U, weight, trace=False):
    assert H.shape == (B, T, D) and U.shape == (B, T, D)
    assert weight.shape == (3 * D,)
    nc = _get_nc()
    in_maps = [
        {
            "H": np.ascontiguousarray(H[b], dtype=np.float32),
            "U": np.ascontiguousarray(U[b], dtype=np.float32),
            "weight": np.ascontiguousarray(weight, dtype=np.float32),
        }
        for b in range(B)
    ]
    res = run_bass_kernel_spmd(nc, in_maps, list(range(B)), trace=trace)
    # device output is S^T per batch
    out = np.stack([res.results[b]["S"].T for b in range(B)], axis=0)
    return out, res


def kernel(H, U, weight):
    out, _ = kernel_with_results(H, U, weight)
    return out


if __name__ == "__main__":
    rng = np.random.default_rng(0)
    H = rng.standard_normal((B, T, D)).astype(np.float32)
    U = rng.standard_normal((B, T, D)).astype(np.float32)
    w = rng.random(3 * D).astype(np.float32)
    out = kernel(H, U, w)
    print(out.shape, out.dtype)
